# revision 15
# baseline (speedup 1.0000x reference)
"""Chamfer loss kernel for 8 Trainium2 NeuronCores.

Strategy
--------
Data parallel over the batch dim: B=16 point clouds, 2 per core.

Host-side (cheap, O(B*K)): compact each cloud to its valid points (the
reference masks invalid rows/cols out of the min with +inf, and the same
mask applies to both sides, so dropping invalid points is exact). Pad to a
common K_p (multiple of 128) with a far-away sentinel point P0=(100,100,100)
shared by pred and target: a padded pred row's nearest target is the padded
target at distance ~0, and no real point ever selects a pad (d2 ~ 3e4, still
finite in fp16); padded rows/columns are excluded from the final sums on the
host.

Device-side (the O(B*K^2) work): for each batch, d2[i,j] is produced by the
TensorEngine as one matmul per tile. fp32 matmuls run at 1/4 rate on TRN2,
so the -2<p,t> + y2 Gram-trick terms are emulated in bf16 with a
split-precision 21-row contraction: each operand is decomposed into three
bf16 terms a0+a1+a2 (~24 mantissa bits total) and the six product pairs
with magnitude >= 2^-18 are separate contraction rows; y2 rides three
ones-rows. Extra contraction rows are free on the PE (<=32 rows stream at
1 column/cycle). The x2[i] term is added EXACTLY in fp32 by the ScalarE
copy (activation bias is per-partition fp32).
    k0-2:a0*b0  k3-5:a0*b1  k6-8:a1*b0  k9-11:a0*b2  k12-14:a1*b1
    k15-17:a2*b0  k18:1*y2_0  k19:1*y2_1  k20:1*y2_2
ScalarE adds x2 and casts each PSUM chunk into one wide [128, K_p] fp16
SBUF tile per row-block. VectorE then does ONE wide tensor_scalar (op0=max(.,1e-12)
clamp, op1=min free-dim reduce at 4x, accum_out -> per-row minima) and ONE
wide tensor_tensor(min) into the per-batch column accumulator (2x). The
column accumulator is reduced across partitions by PE-transposing each
128-wide block (identity matmul) and min-reducing the transposed block's
free dim with tensor_scalar. The host finishes sqrt and sums on the tiny
per-row/per-column minima vectors.
"""

import math

import numpy as np

try:
    import ml_dtypes

    BF16_NP = ml_dtypes.bfloat16
except ImportError:  # pragma: no cover
    BF16_NP = None

import concourse.bass as bass
import concourse.tile as tile
from concourse import mybir
from concourse.bass_utils import run_bass_kernel_spmd

N_CORES = 8
B, K, D = 16, 4096, 3
PAD_COORD = 100.0
BIG = 60000.0
CLAMP = 1.0e-12

F32 = mybir.dt.float32
F16 = mybir.dt.float16
BF16 = mybir.dt.bfloat16
NK = 21  # contraction rows of the split-precision Gram matmul


# ---------------------------------------------------------------------------
# walrus workaround: this build has a single sync-wait slot per instruction.
# Move excess waits onto preceding NOPs on the same engine.
def _split_excess_waits(nc, default_max: int = 1):
    for _bbname, bbobj in list(nc.bb_map.items()):
        inner = bbobj.bb
        insts = inner.instructions
        i = 0
        while i < len(insts):
            inst = insts[i]
            si = inst.sync_info
            if si is not None and si.on_wait and len(si.on_wait) > default_max:
                waits = list(si.on_wait)
                keep, extra = waits[:default_max], waits[default_max:]
                eng = nc.engines[inst.engine]
                new_nops = []
                for w in extra:
                    eng.nop()
                    src = nc.cur_bb.bb.instructions
                    raw = src[-1]
                    assert type(raw).__name__ == "InstNoOp", type(raw).__name__
                    del src[-1]
                    raw.sync_info = mybir.SyncInfo(on_wait=[w], on_update=[])
                    new_nops.append(raw)
                inst.sync_info = mybir.SyncInfo(
                    on_wait=keep, on_update=list(si.on_update or [])
                )
                for j, nop in enumerate(new_nops):
                    insts.insert(i + j, nop)
                i += len(new_nops)
            i += 1


def _chunks_of(width: int):
    """Column chunks: as many 1024-wide (2 PSUM banks) as fit + remainder."""
    out = []
    c0 = 0
    while width - c0 >= 1024:
        out.append((c0, 1024))
        c0 += 1024
    if width - c0 > 0:
        out.append((c0, width - c0))
    return out


def build_nc(K_p: int, n_batches: int = 2):
    RB = K_p // 128
    chunks = _chunks_of(K_p)

    nc = bass.Bass("TRN2", target_bir_lowering=False, debug=False, num_devices=1)

    mats_in = []
    for b in range(n_batches):
        L = nc.dram_tensor(f"L{b}", [NK, K_p], BF16, kind="ExternalInput")
        R = nc.dram_tensor(f"R{b}", [NK, K_p], BF16, kind="ExternalInput")
        mats_in.append((L, R))

    ident_in = nc.dram_tensor("ident", [128, 128], F16, kind="ExternalInput")
    x2s_in = nc.dram_tensor("x2s", [128, n_batches * RB], F32, kind="ExternalInput")
    rowparts_d = nc.dram_tensor(
        "rowparts", [128, n_batches * RB], F32, kind="ExternalOutput"
    )
    colmins_d = nc.dram_tensor(
        "colmins", [128, n_batches * RB], F32, kind="ExternalOutput"
    )

    amax = mybir.AluOpType.max
    amin = mybir.AluOpType.min

    with tile.TileContext(nc) as tc:
        with (
            tc.tile_pool(name="consts", bufs=1) as consts,
            tc.tile_pool(name="work", bufs=4) as work,
            tc.tile_pool(name="psA", bufs=2, space="PSUM") as psA,
            tc.tile_pool(name="psB", bufs=2, space="PSUM") as psB,
        ):
            LR = []
            for b in range(n_batches):
                Lt = consts.tile([NK, K_p], BF16, tag=f"L{b}")
                nc.sync.dma_start(Lt[:], mats_in[b][0].ap())
                Rt = consts.tile([NK, K_p], BF16, tag=f"R{b}")
                nc.sync.dma_start(Rt[:], mats_in[b][1].ap())
                LR.append((Lt, Rt))

            rowparts_sb = consts.tile([128, n_batches * RB], F32, tag="rp")
            colmins_sb = consts.tile([128, n_batches * RB], F32, tag="cm")
            ident = consts.tile([128, 128], F16, tag="ident")
            nc.sync.dma_start(ident[:], ident_in.ap())
            x2s = consts.tile([128, n_batches * RB], F32, tag="x2s")
            nc.sync.dma_start(x2s[:], x2s_in.ap())

            aadd = mybir.AluOpType.add
            for b in range(n_batches):
                Lt, Rt = LR[b]
                colacc = consts.tile([128, K_p], F16, tag=f"colacc{b}")

                for ib in range(RB):
                    lhsT = Lt[:, ib * 128 : (ib + 1) * 128]
                    x2b = x2s[:, b * RB + ib : b * RB + ib + 1]
                    sbw = work.tile([128, K_p], F16, tag="sbw")
                    for ci, (c0, cw) in enumerate(chunks):
                        pool = psA if cw > 512 else psB
                        ps = pool.tile([128, cw], F32, tag=f"ps{cw}")
                        for s in range(0, cw, 512):
                            w = min(512, cw - s)
                            nc.tensor.matmul(
                                ps[:, s : s + w],
                                lhsT,
                                Rt[:, c0 + s : c0 + s + w],
                                start=True,
                                stop=True,
                            )
                        if cw <= 512:
                            # balance: small chunk's copy+bias on DVE
                            nc.vector.tensor_scalar(
                                sbw[:, c0 : c0 + cw], ps[:, :cw], x2b, None, aadd
                            )
                        else:
                            nc.scalar.add(sbw[:, c0 : c0 + cw], ps[:, :cw], x2b)
                    sb2 = work.tile([128, K_p], F16, tag="sb2")
                    idx = b * RB + ib
                    rp = rowparts_sb[:, idx : idx + 1]
                    nc.vector.tensor_scalar(
                        sb2[:], sbw[:], CLAMP, None, amax, amin, accum_out=rp
                    )
                    if ib == 0:
                        nc.vector.tensor_copy(colacc[:], sbw[:])
                    else:
                        nc.vector.tensor_tensor(colacc[:], sbw[:], colacc[:], amin)

                for ib in range(RB):
                    tp = psB.tile([128, 128], F16, tag="trp")
                    nc.tensor.transpose(
                        tp[:], colacc[:, ib * 128 : (ib + 1) * 128], ident[:]
                    )
                    tg = work.tile([128, 128], F16, tag="tg")
                    cm = colmins_sb[:, b * RB + ib : b * RB + ib + 1]
                    nc.vector.tensor_scalar(
                        tg[:], tp[:], CLAMP, None, amax, amin, accum_out=cm
                    )

            nc.sync.dma_start(rowparts_d.ap(), rowparts_sb[:])
            nc.sync.dma_start(colmins_d.ap(), colmins_sb[:])

    _split_excess_waits(nc)
    return nc, RB, chunks


def _split3(a):
    a0 = a.astype(BF16_NP).astype(np.float32)
    a1 = (a - a0).astype(BF16_NP).astype(np.float32)
    a2 = (a - a0 - a1).astype(BF16_NP)
    return a0.astype(BF16_NP), a1.astype(BF16_NP), a2


def _host_prep(pred, target, mask):
    """Compact+pad each batch; build the split-precision NK x K_p matrices."""
    counts = mask.sum(axis=1).astype(np.int64)
    K_p = max(128, int(math.ceil(counts.max() / 128.0)) * 128)

    RB = K_p // 128
    Ls = np.empty((B, NK, K_p), BF16_NP)
    Rs = np.empty((B, NK, K_p), BF16_NP)
    X2s = np.empty((B, 128, RB), np.float32)
    one = np.float32(1.0)
    for b in range(B):
        n = int(counts[b])
        p = np.full((K_p, 3), PAD_COORD, np.float32)
        t = np.full((K_p, 3), PAD_COORD, np.float32)
        p[:n] = pred[b][mask[b]]
        t[:n] = target[b][mask[b]]
        x2 = (p * p).sum(axis=1, dtype=np.float32)
        y2 = (t * t).sum(axis=1, dtype=np.float32)
        a0, a1, a2 = _split3(-2.0 * p)
        b0, b1, b2 = _split3(t)
        y0, y1, y2l = _split3(y2)
        Ls[b, 0:3] = a0.T
        Ls[b, 3:6] = a0.T
        Ls[b, 6:9] = a1.T
        Ls[b, 9:12] = a0.T
        Ls[b, 12:15] = a1.T
        Ls[b, 15:18] = a2.T
        Ls[b, 18] = one
        Ls[b, 19] = one
        Ls[b, 20] = one
        Rs[b, 0:3] = b0.T
        Rs[b, 3:6] = b1.T
        Rs[b, 6:9] = b0.T
        Rs[b, 9:12] = b2.T
        Rs[b, 12:15] = b1.T
        Rs[b, 15:18] = b0.T
        Rs[b, 18] = y0
        Rs[b, 19] = y1
        Rs[b, 20] = y2l
        X2s[b] = x2.reshape(RB, 128).T
    return counts, K_p, Ls, Rs, X2s


_NC_CACHE = {}
_IDENT = np.eye(128, dtype=np.float16)


def kernel(pred, target, mask):
    pred = np.asarray(pred, np.float32)
    target = np.asarray(target, np.float32)
    mask = np.asarray(mask).astype(bool)

    counts, K_p, Ls, Rs, X2s = _host_prep(pred, target, mask)
    nb = B // N_CORES  # batches per core

    key = (K_p, nb)
    if key not in _NC_CACHE:
        _NC_CACHE[key] = build_nc(K_p, nb)
    nc, RB, chunks = _NC_CACHE[key]

    in_maps = []
    for c in range(N_CORES):
        m = {}
        for j in range(nb):
            m[f"L{j}"] = Ls[c * nb + j]
            m[f"R{j}"] = Rs[c * nb + j]
        m["ident"] = _IDENT
        m["x2s"] = np.concatenate(
            [X2s[c * nb + j] for j in range(nb)], axis=1
        )
        in_maps.append(m)

    res = run_bass_kernel_spmd(nc, in_maps, core_ids=list(range(N_CORES)))

    total = np.float32(counts.sum())
    s = np.float64(0.0)
    for c in range(N_CORES):
        rowparts = np.asarray(res.results[c]["rowparts"], np.float32)
        colmins = np.asarray(res.results[c]["colmins"], np.float32)
        for j in range(nb):
            n = int(counts[c * nb + j])
            # row r = ib*128+p lives at rowparts[p, j*RB+ib]
            rowmin = rowparts[:, j * RB : (j + 1) * RB].T.reshape(-1)[:n]
            s += np.sqrt(np.maximum(rowmin, CLAMP), dtype=np.float32).sum(
                dtype=np.float64
            )
            ct = colmins[:, j * RB : (j + 1) * RB].T.reshape(-1)[:n]
            s += np.sqrt(np.maximum(ct, CLAMP), dtype=np.float32).sum(
                dtype=np.float64
            )

    loss = s / (2.0 * (np.float64(total) + 1e-8))
    return np.float32(loss)


# revision 16
# speedup vs baseline: 1.3343x; 1.3343x over previous
"""Chamfer loss kernel for 8 Trainium2 NeuronCores.

Strategy
--------
Data parallel over the batch dim: B=16 point clouds, 2 per core.

Host-side (cheap, O(B*K)): compact each cloud to its valid points (the
reference masks invalid rows/cols out of the min with +inf, and the same
mask applies to both sides, so dropping invalid points is exact). Pad to a
common K_p (multiple of 128) with a far-away sentinel point P0=(100,100,100)
shared by pred and target: a padded pred row's nearest target is the padded
target at distance ~0, and no real point ever selects a pad (d2 ~ 3e4, still
finite in fp16); padded rows/columns are excluded from the final sums on the
host.

Device-side (the O(B*K^2) work): for each batch, d2[i,j] is produced by the
TensorEngine as one matmul per tile. fp32 matmuls run at 1/4 rate on TRN2,
so the -2<p,t> + y2 Gram-trick terms are emulated in bf16 with a
split-precision 21-row contraction: each operand is decomposed into three
bf16 terms a0+a1+a2 (~24 mantissa bits total) and the six product pairs
with magnitude >= 2^-18 are separate contraction rows; y2 rides three
ones-rows. Extra contraction rows are free on the PE (<=32 rows stream at
1 column/cycle). The x2[i] term is added EXACTLY in fp32 by the ScalarE
copy (activation bias is per-partition fp32).
    k0-2:a0*b0  k3-5:a0*b1  k6-8:a1*b0  k9-11:a0*b2  k12-14:a1*b1
    k15-17:a2*b0  k18:1*y2_0  k19:1*y2_1  k20:1*y2_2
ScalarE adds x2 and casts each PSUM chunk into one wide [128, K_p] fp16
SBUF tile per row-block. VectorE then does ONE wide tensor_scalar (op0=max(.,1e-12)
clamp, op1=min free-dim reduce at 4x, accum_out -> per-row minima) and ONE
wide tensor_tensor(min) into the per-batch column accumulator (2x). The
column accumulator is reduced across partitions by PE-transposing each
128-wide block (identity matmul) and min-reducing the transposed block's
free dim with tensor_scalar. The host finishes sqrt and sums on the tiny
per-row/per-column minima vectors.
"""

import math

import numpy as np

try:
    import ml_dtypes

    BF16_NP = ml_dtypes.bfloat16
except ImportError:  # pragma: no cover
    BF16_NP = None

import concourse.bass as bass
import concourse.tile as tile
from concourse import mybir
from concourse.bass_utils import run_bass_kernel_spmd

N_CORES = 8
B, K, D = 16, 4096, 3
PAD_COORD = 100.0
BIG = 60000.0
CLAMP = 1.0e-12

F32 = mybir.dt.float32
F16 = mybir.dt.float16
BF16 = mybir.dt.bfloat16
NK = 21  # contraction rows of the split-precision Gram matmul


# ---------------------------------------------------------------------------
# walrus workaround: this build has a single sync-wait slot per instruction.
# Move excess waits onto preceding NOPs on the same engine.
def _split_excess_waits(nc, default_max: int = 1):
    for _bbname, bbobj in list(nc.bb_map.items()):
        inner = bbobj.bb
        insts = inner.instructions
        i = 0
        while i < len(insts):
            inst = insts[i]
            si = inst.sync_info
            if si is not None and si.on_wait and len(si.on_wait) > default_max:
                waits = list(si.on_wait)
                keep, extra = waits[:default_max], waits[default_max:]
                eng = nc.engines[inst.engine]
                new_nops = []
                for w in extra:
                    eng.nop()
                    src = nc.cur_bb.bb.instructions
                    raw = src[-1]
                    assert type(raw).__name__ == "InstNoOp", type(raw).__name__
                    del src[-1]
                    raw.sync_info = mybir.SyncInfo(on_wait=[w], on_update=[])
                    new_nops.append(raw)
                inst.sync_info = mybir.SyncInfo(
                    on_wait=keep, on_update=list(si.on_update or [])
                )
                for j, nop in enumerate(new_nops):
                    insts.insert(i + j, nop)
                i += len(new_nops)
            i += 1


def _chunks_of(width: int):
    """Column chunks: as many 1024-wide (2 PSUM banks) as fit + remainder."""
    out = []
    c0 = 0
    while width - c0 >= 1024:
        out.append((c0, 1024))
        c0 += 1024
    if width - c0 > 0:
        out.append((c0, width - c0))
    return out


def build_nc(K_p: int, n_batches: int = 2):
    RB = K_p // 128
    chunks = _chunks_of(K_p)

    nc = bass.Bass("TRN2", target_bir_lowering=False, debug=False, num_devices=1)

    mats_in = []
    for b in range(n_batches):
        L = nc.dram_tensor(f"L{b}", [NK, K_p], BF16, kind="ExternalInput")
        R = nc.dram_tensor(f"R{b}", [NK, K_p], BF16, kind="ExternalInput")
        mats_in.append((L, R))

    ident_in = nc.dram_tensor("ident", [128, 128], F16, kind="ExternalInput")
    x2s_in = nc.dram_tensor("x2s", [128, n_batches * RB], F32, kind="ExternalInput")
    rowparts_d = nc.dram_tensor(
        "rowparts", [128, n_batches * RB], F32, kind="ExternalOutput"
    )
    colmins_d = nc.dram_tensor(
        "colmins", [128, n_batches * RB], F32, kind="ExternalOutput"
    )

    amax = mybir.AluOpType.max
    amin = mybir.AluOpType.min

    with tile.TileContext(nc) as tc:
        with (
            tc.tile_pool(name="consts", bufs=1) as consts,
            tc.tile_pool(name="work", bufs=4) as work,
            tc.tile_pool(name="psA", bufs=2, space="PSUM") as psA,
            tc.tile_pool(name="psB", bufs=2, space="PSUM") as psB,
        ):
            LR = []
            for b in range(n_batches):
                Lt = consts.tile([NK, K_p], BF16, tag=f"L{b}")
                nc.sync.dma_start(Lt[:], mats_in[b][0].ap())
                Rt = consts.tile([NK, K_p], BF16, tag=f"R{b}")
                nc.sync.dma_start(Rt[:], mats_in[b][1].ap())
                LR.append((Lt, Rt))

            rowparts_sb = consts.tile([128, n_batches * RB], F32, tag="rp")
            colmins_sb = consts.tile([128, n_batches * RB], F32, tag="cm")
            ident = consts.tile([128, 128], F16, tag="ident")
            nc.sync.dma_start(ident[:], ident_in.ap())
            x2s = consts.tile([128, n_batches * RB], F32, tag="x2s")
            nc.sync.dma_start(x2s[:], x2s_in.ap())

            aadd = mybir.AluOpType.add
            for b in range(n_batches):
                Lt, Rt = LR[b]
                colacc = consts.tile([128, K_p], F16, tag=f"colacc{b}")

                for ib in range(RB):
                    lhsT = Lt[:, ib * 128 : (ib + 1) * 128]
                    x2b = x2s[:, b * RB + ib : b * RB + ib + 1]
                    sbw = work.tile([128, K_p], F16, tag="sbw")
                    for ci, (c0, cw) in enumerate(chunks):
                        pool = psA if cw > 512 else psB
                        ps = pool.tile([128, cw], F32, tag=f"ps{cw}")
                        for s in range(0, cw, 512):
                            w = min(512, cw - s)
                            nc.tensor.matmul(
                                ps[:, s : s + w],
                                lhsT,
                                Rt[:, c0 + s : c0 + s + w],
                                start=True,
                                stop=True,
                            )
                        if cw <= 512:
                            # balance: small chunk's copy+bias on DVE
                            nc.vector.tensor_scalar(
                                sbw[:, c0 : c0 + cw], ps[:, :cw], x2b, None, aadd
                            )
                        else:
                            nc.scalar.add(sbw[:, c0 : c0 + cw], ps[:, :cw], x2b)
                    sb2 = work.tile([128, K_p], F16, tag="sb2")
                    idx = b * RB + ib
                    rp = rowparts_sb[:, idx : idx + 1]
                    nc.vector.tensor_scalar(
                        sb2[:], sbw[:], CLAMP, None, amax, amin, accum_out=rp
                    )
                    if ib == 0:
                        nc.vector.tensor_copy(colacc[:], sbw[:])
                    else:
                        nc.vector.tensor_tensor(colacc[:], sbw[:], colacc[:], amin)

                for g0 in range(0, RB, 4):
                    gn = min(4, RB - g0)
                    tp = psB.tile([128, 4 * 128], F16, tag="trp")
                    for k in range(gn):
                        ib = g0 + k
                        nc.tensor.transpose(
                            tp[:, k * 128 : (k + 1) * 128],
                            colacc[:, ib * 128 : (ib + 1) * 128],
                            ident[:],
                        )
                    cm = colmins_sb[:, b * RB + g0 : b * RB + g0 + gn]
                    nc.vector.tensor_reduce(
                        cm,
                        tp[:, : gn * 128].rearrange("p (g q) -> p g q", q=128),
                        axis=mybir.AxisListType.X,
                        op=amin,
                    )

            nc.sync.dma_start(rowparts_d.ap(), rowparts_sb[:])
            nc.sync.dma_start(colmins_d.ap(), colmins_sb[:])

    _split_excess_waits(nc)
    return nc, RB, chunks


def _split3(a):
    a0 = a.astype(BF16_NP).astype(np.float32)
    a1 = (a - a0).astype(BF16_NP).astype(np.float32)
    a2 = (a - a0 - a1).astype(BF16_NP)
    return a0.astype(BF16_NP), a1.astype(BF16_NP), a2


def _host_prep(pred, target, mask):
    """Compact+pad each batch; build the split-precision NK x K_p matrices."""
    counts = mask.sum(axis=1).astype(np.int64)
    K_p = max(128, int(math.ceil(counts.max() / 128.0)) * 128)

    RB = K_p // 128
    Ls = np.empty((B, NK, K_p), BF16_NP)
    Rs = np.empty((B, NK, K_p), BF16_NP)
    X2s = np.empty((B, 128, RB), np.float32)
    one = np.float32(1.0)
    for b in range(B):
        n = int(counts[b])
        p = np.full((K_p, 3), PAD_COORD, np.float32)
        t = np.full((K_p, 3), PAD_COORD, np.float32)
        p[:n] = pred[b][mask[b]]
        t[:n] = target[b][mask[b]]
        x2 = (p * p).sum(axis=1, dtype=np.float32)
        y2 = (t * t).sum(axis=1, dtype=np.float32)
        a0, a1, a2 = _split3(-2.0 * p)
        b0, b1, b2 = _split3(t)
        y0, y1, y2l = _split3(y2)
        Ls[b, 0:3] = a0.T
        Ls[b, 3:6] = a0.T
        Ls[b, 6:9] = a1.T
        Ls[b, 9:12] = a0.T
        Ls[b, 12:15] = a1.T
        Ls[b, 15:18] = a2.T
        Ls[b, 18] = one
        Ls[b, 19] = one
        Ls[b, 20] = one
        Rs[b, 0:3] = b0.T
        Rs[b, 3:6] = b1.T
        Rs[b, 6:9] = b0.T
        Rs[b, 9:12] = b2.T
        Rs[b, 12:15] = b1.T
        Rs[b, 15:18] = b0.T
        Rs[b, 18] = y0
        Rs[b, 19] = y1
        Rs[b, 20] = y2l
        X2s[b] = x2.reshape(RB, 128).T
    return counts, K_p, Ls, Rs, X2s


_NC_CACHE = {}
_IDENT = np.eye(128, dtype=np.float16)


def kernel(pred, target, mask):
    pred = np.asarray(pred, np.float32)
    target = np.asarray(target, np.float32)
    mask = np.asarray(mask).astype(bool)

    counts, K_p, Ls, Rs, X2s = _host_prep(pred, target, mask)
    nb = B // N_CORES  # batches per core

    key = (K_p, nb)
    if key not in _NC_CACHE:
        _NC_CACHE[key] = build_nc(K_p, nb)
    nc, RB, chunks = _NC_CACHE[key]

    in_maps = []
    for c in range(N_CORES):
        m = {}
        for j in range(nb):
            m[f"L{j}"] = Ls[c * nb + j]
            m[f"R{j}"] = Rs[c * nb + j]
        m["ident"] = _IDENT
        m["x2s"] = np.concatenate(
            [X2s[c * nb + j] for j in range(nb)], axis=1
        )
        in_maps.append(m)

    res = run_bass_kernel_spmd(nc, in_maps, core_ids=list(range(N_CORES)))

    total = np.float32(counts.sum())
    s = np.float64(0.0)
    for c in range(N_CORES):
        rowparts = np.asarray(res.results[c]["rowparts"], np.float32)
        colmins = np.asarray(res.results[c]["colmins"], np.float32)
        for j in range(nb):
            n = int(counts[c * nb + j])
            # row r = ib*128+p lives at rowparts[p, j*RB+ib]
            rowmin = rowparts[:, j * RB : (j + 1) * RB].T.reshape(-1)[:n]
            s += np.sqrt(np.maximum(rowmin, CLAMP), dtype=np.float32).sum(
                dtype=np.float64
            )
            ct = colmins[:, j * RB : (j + 1) * RB].T.reshape(-1)[:n]
            s += np.sqrt(np.maximum(ct, CLAMP), dtype=np.float32).sum(
                dtype=np.float64
            )

    loss = s / (2.0 * (np.float64(total) + 1e-8))
    return np.float32(loss)


# revision 20
# speedup vs baseline: 4811.9733x; 3606.2574x over previous
"""Chamfer loss kernel for 8 Trainium2 NeuronCores.

Strategy
--------
Data parallel over the batch dim: B=16 point clouds, 2 per core.

Host-side (cheap, O(B*K)): compact each cloud to its valid points (the
reference masks invalid rows/cols out of the min with +inf, and the same
mask applies to both sides, so dropping invalid points is exact). Pad to a
common K_p (multiple of 128) with a far-away sentinel point P0=(100,100,100)
shared by pred and target: a padded pred row's nearest target is the padded
target at distance ~0, and no real point ever selects a pad (d2 ~ 3e4, still
finite in fp16); padded rows/columns are excluded from the final sums on the
host.

Device-side (the O(B*K^2) work): for each batch, d2[i,j] is produced by the
TensorEngine as one matmul per tile. fp32 matmuls run at 1/4 rate on TRN2,
so the -2<p,t> + y2 Gram-trick terms are emulated in bf16 with a
split-precision 21-row contraction: each operand is decomposed into three
bf16 terms a0+a1+a2 (~24 mantissa bits total) and the six product pairs
with magnitude >= 2^-18 are separate contraction rows; y2 rides three
ones-rows. Extra contraction rows are free on the PE (<=32 rows stream at
1 column/cycle). The x2[i] term is added EXACTLY in fp32 by the ScalarE
copy (activation bias is per-partition fp32).
    k0-2:a0*b0  k3-5:a0*b1  k6-8:a1*b0  k9-11:a0*b2  k12-14:a1*b1
    k15-17:a2*b0  k18:1*y2_0  k19:1*y2_1  k20:1*y2_2
ScalarE adds x2 and casts each PSUM chunk into one wide [128, K_p] fp16
SBUF tile per row-block. VectorE then does ONE wide tensor_scalar (op0=max(.,1e-12)
clamp, op1=min free-dim reduce at 4x, accum_out -> per-row minima) and ONE
wide tensor_tensor(min) into the per-batch column accumulator (2x). The
column accumulator is reduced across partitions by PE-transposing each
128-wide block (identity matmul) and min-reducing the transposed block's
free dim with tensor_scalar. The host finishes sqrt and sums on the tiny
per-row/per-column minima vectors.
"""

import math

import numpy as np

try:
    import ml_dtypes

    BF16_NP = ml_dtypes.bfloat16
except ImportError:  # pragma: no cover
    BF16_NP = None

import concourse.bass as bass
import concourse.tile as tile
from concourse import mybir
from concourse.bass_utils import run_bass_kernel_spmd

N_CORES = 8
B, K, D = 16, 4096, 3
PAD_COORD = 100.0
BIG = 60000.0
CLAMP = 1.0e-12

F32 = mybir.dt.float32
F16 = mybir.dt.float16
BF16 = mybir.dt.bfloat16
NK = 21  # contraction rows of the split-precision Gram matmul


# ---------------------------------------------------------------------------
# walrus workaround: this build has a single sync-wait slot per instruction.
# Move excess waits onto preceding NOPs on the same engine.
def _split_excess_waits(nc, default_max: int = 1):
    for _bbname, bbobj in list(nc.bb_map.items()):
        inner = bbobj.bb
        insts = inner.instructions
        i = 0
        while i < len(insts):
            inst = insts[i]
            si = inst.sync_info
            if si is not None and si.on_wait and len(si.on_wait) > default_max:
                waits = list(si.on_wait)
                keep, extra = waits[:default_max], waits[default_max:]
                eng = nc.engines[inst.engine]
                new_nops = []
                for w in extra:
                    eng.nop()
                    src = nc.cur_bb.bb.instructions
                    raw = src[-1]
                    assert type(raw).__name__ == "InstNoOp", type(raw).__name__
                    del src[-1]
                    raw.sync_info = mybir.SyncInfo(on_wait=[w], on_update=[])
                    new_nops.append(raw)
                inst.sync_info = mybir.SyncInfo(
                    on_wait=keep, on_update=list(si.on_update or [])
                )
                for j, nop in enumerate(new_nops):
                    insts.insert(i + j, nop)
                i += len(new_nops)
            i += 1


def _chunks_of(width: int):
    """Column chunks: as many 1024-wide (2 PSUM banks) as fit + remainder."""
    out = []
    c0 = 0
    while width - c0 >= 1024:
        out.append((c0, 1024))
        c0 += 1024
    if width - c0 > 0:
        out.append((c0, width - c0))
    return out


def build_nc(K_p: int, n_batches: int = 2):
    RB = K_p // 128
    chunks = _chunks_of(K_p)

    nc = bass.Bass("TRN2", target_bir_lowering=False, debug=False, num_devices=1)

    mats_in = []
    for b in range(n_batches):
        L = nc.dram_tensor(f"L{b}", [NK, K_p], BF16, kind="ExternalInput")
        R = nc.dram_tensor(f"R{b}", [NK, K_p], BF16, kind="ExternalInput")
        mats_in.append((L, R))

    ident_in = nc.dram_tensor("ident", [128, 128], F16, kind="ExternalInput")
    x2s_in = nc.dram_tensor("x2s", [128, n_batches * RB], F32, kind="ExternalInput")
    rowparts_d = nc.dram_tensor(
        "rowparts", [128, n_batches * RB], F32, kind="ExternalOutput"
    )
    colmins_d = nc.dram_tensor(
        "colmins", [128, n_batches * RB], F32, kind="ExternalOutput"
    )

    amax = mybir.AluOpType.max
    amin = mybir.AluOpType.min

    with tile.TileContext(nc) as tc:
        with (
            tc.tile_pool(name="consts", bufs=1) as consts,
            tc.tile_pool(name="work", bufs=4) as work,
            tc.tile_pool(name="psA", bufs=2, space="PSUM") as psA,
            tc.tile_pool(name="psB", bufs=2, space="PSUM") as psB,
        ):
            LR = []
            for b in range(n_batches):
                Lt = consts.tile([NK, K_p], BF16, tag=f"L{b}")
                nc.sync.dma_start(Lt[:], mats_in[b][0].ap())
                Rt = consts.tile([NK, K_p], BF16, tag=f"R{b}")
                nc.sync.dma_start(Rt[:], mats_in[b][1].ap())
                LR.append((Lt, Rt))

            rowparts_sb = consts.tile([128, n_batches * RB], F32, tag="rp")
            colmins_sb = consts.tile([128, n_batches * RB], F32, tag="cm")
            ident = consts.tile([128, 128], F16, tag="ident")
            nc.sync.dma_start(ident[:], ident_in.ap())
            x2s = consts.tile([128, n_batches * RB], F32, tag="x2s")
            nc.sync.dma_start(x2s[:], x2s_in.ap())

            aadd = mybir.AluOpType.add
            for b in range(n_batches):
                Lt, Rt = LR[b]
                colacc = consts.tile([128, K_p], F16, tag=f"colacc{b}")

                for ib in range(RB):
                    lhsT = Lt[:, ib * 128 : (ib + 1) * 128]
                    x2b = x2s[:, b * RB + ib : b * RB + ib + 1]
                    sbw = work.tile([128, K_p], F16, tag="sbw")
                    for ci, (c0, cw) in enumerate(chunks):
                        if cw <= 512:
                            psfull = psB.tile([128, cw], F32, tag="psR")
                        else:
                            psfull = psA.tile([128, 1024], F32, tag="ps")
                        ps = psfull[:, :cw]
                        for s in range(0, cw, 512):
                            w = min(512, cw - s)
                            nc.tensor.matmul(
                                ps[:, s : s + w],
                                lhsT,
                                Rt[:, c0 + s : c0 + s + w],
                                start=True,
                                stop=True,
                            )
                        if cw <= 512:
                            # balance: small chunk's copy+bias on DVE
                            nc.vector.tensor_scalar(
                                sbw[:, c0 : c0 + cw], ps[:], x2b, None, aadd
                            )
                        else:
                            nc.scalar.add(sbw[:, c0 : c0 + cw], ps[:], x2b)
                    sb2 = work.tile([128, K_p], F16, tag="sb2")
                    idx = b * RB + ib
                    rp = rowparts_sb[:, idx : idx + 1]
                    nc.vector.tensor_scalar(
                        sb2[:], sbw[:], CLAMP, None, amax, amin, accum_out=rp
                    )
                    if ib == 0:
                        nc.vector.tensor_copy(colacc[:], sbw[:])
                    else:
                        nc.vector.tensor_tensor(colacc[:], sbw[:], colacc[:], amin)

                for g0 in range(0, RB, 4):
                    gn = min(4, RB - g0)
                    tp = psB.tile([128, 4 * 128], F16, tag="trp")
                    for k in range(gn):
                        ib = g0 + k
                        nc.tensor.transpose(
                            tp[:, k * 128 : (k + 1) * 128],
                            colacc[:, ib * 128 : (ib + 1) * 128],
                            ident[:],
                        )
                    cm = colmins_sb[:, b * RB + g0 : b * RB + g0 + gn]
                    nc.vector.tensor_reduce(
                        cm,
                        tp[:, : gn * 128].rearrange("p (g q) -> p g q", q=128),
                        axis=mybir.AxisListType.X,
                        op=amin,
                    )

            nc.sync.dma_start(rowparts_d.ap(), rowparts_sb[:])
            nc.sync.dma_start(colmins_d.ap(), colmins_sb[:])

    _split_excess_waits(nc)
    return nc, RB, chunks


def _split3(a):
    a0 = a.astype(BF16_NP).astype(np.float32)
    a1 = (a - a0).astype(BF16_NP).astype(np.float32)
    a2 = (a - a0 - a1).astype(BF16_NP)
    return a0.astype(BF16_NP), a1.astype(BF16_NP), a2


def _host_prep(pred, target, mask):
    """Compact+pad each batch; build the split-precision NK x K_p matrices."""
    counts = mask.sum(axis=1).astype(np.int64)
    K_p = max(128, int(math.ceil(counts.max() / 128.0)) * 128)

    RB = K_p // 128
    Ls = np.empty((B, NK, K_p), BF16_NP)
    Rs = np.empty((B, NK, K_p), BF16_NP)
    X2s = np.empty((B, 128, RB), np.float32)
    one = np.float32(1.0)
    for b in range(B):
        n = int(counts[b])
        p = np.full((K_p, 3), PAD_COORD, np.float32)
        t = np.full((K_p, 3), PAD_COORD, np.float32)
        p[:n] = pred[b][mask[b]]
        t[:n] = target[b][mask[b]]
        x2 = (p * p).sum(axis=1, dtype=np.float32)
        y2 = (t * t).sum(axis=1, dtype=np.float32)
        a0, a1, a2 = _split3(-2.0 * p)
        b0, b1, b2 = _split3(t)
        y0, y1, y2l = _split3(y2)
        Ls[b, 0:3] = a0.T
        Ls[b, 3:6] = a0.T
        Ls[b, 6:9] = a1.T
        Ls[b, 9:12] = a0.T
        Ls[b, 12:15] = a1.T
        Ls[b, 15:18] = a2.T
        Ls[b, 18] = one
        Ls[b, 19] = one
        Ls[b, 20] = one
        Rs[b, 0:3] = b0.T
        Rs[b, 3:6] = b1.T
        Rs[b, 6:9] = b0.T
        Rs[b, 9:12] = b2.T
        Rs[b, 12:15] = b1.T
        Rs[b, 15:18] = b0.T
        Rs[b, 18] = y0
        Rs[b, 19] = y1
        Rs[b, 20] = y2l
        X2s[b] = x2.reshape(RB, 128).T
    return counts, K_p, Ls, Rs, X2s


_NC_CACHE = {}
_IDENT = np.eye(128, dtype=np.float16)


def kernel(pred, target, mask):
    pred = np.asarray(pred, np.float32)
    target = np.asarray(target, np.float32)
    mask = np.asarray(mask).astype(bool)

    counts, K_p, Ls, Rs, X2s = _host_prep(pred, target, mask)
    nb = B // N_CORES  # batches per core

    key = (K_p, nb)
    if key not in _NC_CACHE:
        _NC_CACHE[key] = build_nc(K_p, nb)
    nc, RB, chunks = _NC_CACHE[key]

    in_maps = []
    for c in range(N_CORES):
        m = {}
        for j in range(nb):
            m[f"L{j}"] = Ls[c * nb + j]
            m[f"R{j}"] = Rs[c * nb + j]
        m["ident"] = _IDENT
        m["x2s"] = np.concatenate(
            [X2s[c * nb + j] for j in range(nb)], axis=1
        )
        in_maps.append(m)

    res = run_bass_kernel_spmd(nc, in_maps, core_ids=list(range(N_CORES)))

    total = np.float32(counts.sum())
    s = np.float64(0.0)
    for c in range(N_CORES):
        rowparts = np.asarray(res.results[c]["rowparts"], np.float32)
        colmins = np.asarray(res.results[c]["colmins"], np.float32)
        for j in range(nb):
            n = int(counts[c * nb + j])
            # row r = ib*128+p lives at rowparts[p, j*RB+ib]
            rowmin = rowparts[:, j * RB : (j + 1) * RB].T.reshape(-1)[:n]
            s += np.sqrt(np.maximum(rowmin, CLAMP), dtype=np.float32).sum(
                dtype=np.float64
            )
            ct = colmins[:, j * RB : (j + 1) * RB].T.reshape(-1)[:n]
            s += np.sqrt(np.maximum(ct, CLAMP), dtype=np.float32).sum(
                dtype=np.float64
            )

    loss = s / (2.0 * (np.float64(total) + 1e-8))
    return np.float32(loss)
